# revision 1
# baseline (speedup 1.0000x reference)
"""Distributed CrossAttention (self-attention) kernel for 8 TRN2 NeuronCores.

Problem: B=2, S=2048, D=1024, H=16, DH=64, fp32.
  q/k/v = x@W.T + b; RMSNorm(q/k over full D); RoPE; SDPA; out-proj.

Sharding: core c -> (batch b = c//4, head-group g = c%4) = 4 heads = 256 dims.
Everything on-device is feature-major ("transposed"): the host pre-transposes x
and the weight slices (packed into wide contiguous rows for DMA descriptor
efficiency), so no on-device transposes are needed.

Per-core pipeline:
  1. Q then K e-major projections ([256,S], fp32r matmuls, W-stationary) with
     qn_w/kn_w folded into the weights on the host; per-token sum-of-squares of
     the raw q/k via a g=1/w^2 column matmul; one small AllReduce per tensor
     over the 4-core batch group (hidden under the V pass).
  2. V projection t-major (x-stationary) into a padded [t, 4*(64+1)] layout
     whose 65th column is ones - the softmax denominator rides along the AV
     matmul for free.
  3. RoPE as A*q + B*(P@q): pair-swap permutation matmul + host-prepared
     e-major pe coefficient planes A/B; k first (no AR dependency), then q
     with rs_q = 1/sqrt(mean(q^2)+eps) folded in (gpsimd partition_broadcast).
  4. SDPA per head pair in bf16: scoresT = k_h.T @ q_h with K=64 PE row-tiling
     (partition bases 0/64); exp on ScalarE straight out of PSUM [128,1024]
     with rs_k/sqrt(DH) as its per-partition scale operand; AV accumulates
     over j-chunks; divide via reciprocal + a K=1 ones-matmul broadcast.
  5. Per-i-tile bf16 AllGather of attention output over the batch group
     (pipelined with the next i-tile's attention), then the local Wo slice
     with bias fused into the PSUM eviction.
"""
import numpy as np
from contextlib import ExitStack

import concourse.bass as bass
import concourse.mybir as mybir
import concourse.tile as tile
import concourse.bacc as bacc
from concourse.bass_utils import run_bass_kernel_spmd

F32 = mybir.dt.float32
F32R = mybir.dt.float32r
BF16 = mybir.dt.bfloat16
AF = mybir.ActivationFunctionType
MUL = mybir.AluOpType.mult

B, S, D, H, DH = 2, 2048, 1024, 16, 64
EPS = 1e-5
N_CORES = 8
EL = 256            # e-dims per core
NHL = 4             # heads per core
TC = 512            # token chunk (matmul moving N)
NTC = S // TC       # 4
NDC = D // 128      # 8 contraction chunks
NJC = S // 128      # 16 key chunks
GROUPS = [[0, 1, 2, 3], [4, 5, 6, 7]]

TRACE = False       # test.py flips this for profiling


def _emit_rope(nc, tc, ab_pool, tmp_pool, psw, perm_sb, ra, rb, RSq, srct, rot, et, is_q):
    esl = slice(128 * et, 128 * et + 128)
    for tcix in range(NTC):
        tsl = slice(TC * tcix, TC * tcix + TC)
        at = ab_pool.tile([128, TC], F32, tag="ab", name="at")
        bt = ab_pool.tile([128, TC], F32, tag="ab", name="bt")
        nc.sync.dma_start(at[:], ra[esl, tsl])
        nc.sync.dma_start(bt[:], rb[esl, tsl])
        swp = psw.tile([128, TC], F32, tag="psw")
        nc.tensor.matmul(swp[:], perm_sb[:], srct[et][:, tsl], start=True, stop=True)
        t1 = tmp_pool.tile([128, TC], F32, tag="tmp")
        nc.vector.tensor_tensor(t1[:], at[:], srct[et][:, tsl], MUL)
        t2 = tmp_pool.tile([128, TC], F32, tag="tmp")
        nc.vector.tensor_tensor(t2[:], bt[:], swp[:], MUL)
        if is_q:
            t3 = tmp_pool.tile([128, TC], F32, tag="tmp")
            nc.vector.tensor_add(t3[:], t1[:], t2[:])
            nc.vector.tensor_tensor(rot[et][:, tsl], t3[:], RSq[:, tsl], MUL)
        else:
            nc.vector.tensor_add(rot[et][:, tsl], t1[:], t2[:])


def _emit(nc):
    xT = nc.declare_dram_parameter("xT", [D, S], F32, isOutput=False)
    wq = nc.declare_dram_parameter("wq", [128, NDC * EL], F32, isOutput=False)
    wk = nc.declare_dram_parameter("wk", [128, NDC * EL], F32, isOutput=False)
    wv = nc.declare_dram_parameter("wv", [128, NDC * EL], F32, isOutput=False)
    wo = nc.declare_dram_parameter("wo", [128, NDC * EL], F32, isOutput=False)
    bq = nc.declare_dram_parameter("bq", [EL, 1], F32, isOutput=False)
    bk = nc.declare_dram_parameter("bk", [EL, 1], F32, isOutput=False)
    bv = nc.declare_dram_parameter("bv", [1, EL], F32, isOutput=False)
    bo = nc.declare_dram_parameter("bo", [EL, 1], F32, isOutput=False)
    gq = nc.declare_dram_parameter("gq", [EL, 1], F32, isOutput=False)
    gk = nc.declare_dram_parameter("gk", [EL, 1], F32, isOutput=False)
    ra = nc.declare_dram_parameter("rope_a", [EL, S], F32, isOutput=False)
    rb = nc.declare_dram_parameter("rope_b", [EL, S], F32, isOutput=False)
    pm = nc.declare_dram_parameter("perm", [128, 128], F32, isOutput=False)
    yT = nc.declare_dram_parameter("yT", [EL, S], F32, isOutput=True)

    with tile.TileContext(nc) as tc, ExitStack() as ctx:
        # ---------------- persistent pools ----------------
        w_pool = ctx.enter_context(tc.tile_pool(name="w", bufs=1))
        qk_pool = ctx.enter_context(tc.tile_pool(name="qk", bufs=4))
        v_pool = ctx.enter_context(tc.tile_pool(name="v", bufs=16))
        small = ctx.enter_context(tc.tile_pool(name="small", bufs=1))
        rsq_pool = ctx.enter_context(tc.tile_pool(name="rsq", bufs=1))
        dram = ctx.enter_context(tc.tile_pool(name="dram", bufs=16, space="DRAM"))

        # ---------------- constants / small loads ----------------
        bq_sb, bk_sb, bo_sb, gq_sb, gk_sb = [], [], [], [], []
        for et in range(2):
            sl = slice(128 * et, 128 * et + 128)
            t = small.tile([128, 1], F32, tag=f"bq{et}", name=f"bq{et}")
            nc.sync.dma_start(t[:], bq[sl, :]); bq_sb.append(t)
            t = small.tile([128, 1], F32, tag=f"bk{et}", name=f"bk{et}")
            nc.sync.dma_start(t[:], bk[sl, :]); bk_sb.append(t)
            t = small.tile([128, 1], F32, tag=f"bo{et}", name=f"bo{et}")
            nc.sync.dma_start(t[:], bo[sl, :]); bo_sb.append(t)
            t = small.tile([128, 1], F32R, tag=f"gq{et}", name=f"gq{et}")
            nc.sync.dma_start(t[:], gq[sl, :].bitcast(F32R)); gq_sb.append(t)
            t = small.tile([128, 1], F32R, tag=f"gk{et}", name=f"gk{et}")
            nc.sync.dma_start(t[:], gk[sl, :].bitcast(F32R)); gk_sb.append(t)
        bv_sb = small.tile([1, EL], F32R, tag="bvrow")
        nc.sync.dma_start(bv_sb[:], bv[:].bitcast(F32R))
        ones1 = small.tile([1, 128], F32, tag="ones1")
        nc.vector.memset(ones1[:], 1.0)
        ones1r = small.tile([1, 128], F32R, tag="ones1r")
        nc.vector.tensor_copy(ones1r[:], ones1[:])
        ones4 = small.tile([128, 4], F32, tag="ones4")
        nc.vector.memset(ones4[:], 1.0)
        eps_t = small.tile([2, 1], F32, tag="eps")
        nc.vector.memset(eps_t[:], EPS)
        perm_sb = small.tile([128, 128], F32R, tag="perm")
        nc.sync.dma_start(perm_sb[:], pm[:].bitcast(F32R))

        wq_all = w_pool.tile([128, NDC * EL], F32R, tag="wq", name="wq_all", bufs=1)
        wk_all = w_pool.tile([128, NDC * EL], F32R, tag="wk", name="wk_all", bufs=1)
        wv_all = w_pool.tile([128, NDC * EL], F32R, tag="wv", name="wv_all", bufs=1)
        wq_sb = [wq_all[:, EL * i:EL * i + EL] for i in range(NDC)]
        wk_sb = [wk_all[:, EL * i:EL * i + EL] for i in range(NDC)]
        wv_sb = [wv_all[:, EL * i:EL * i + EL] for i in range(NDC)]
        nc.sync.dma_start(wq_all[:], wq[:].bitcast(F32R))

        # persistent activation tiles
        qT_sb = [qk_pool.tile([128, S], F32R, tag="qk", name=f"qT_sb{i}") for i in range(2)]
        kT_sb = [qk_pool.tile([128, S], F32R, tag="qk", name=f"kT_sb{i}") for i in range(2)]
        v_sb = [v_pool.tile([128, NHL * 65], BF16, tag="v", name=f"v_sb{i}") for i in range(NJC)]
        for jt in range(NJC):
            ones_cols = v_sb[jt][:].rearrange("p (h c) -> p h c", c=65)[:, :, 64:65]
            nc.vector.tensor_copy(
                ones_cols, ones4[:].rearrange("p (f o) -> p f o", o=1)
            )
        RSq = rsq_pool.tile([128, S], F32, tag="rsq")
        rskT = small.tile([128, NJC], F32, tag="rskT")

        # ---------------- phase A: projections + ssq + rope ----------------
        rot_q = [None, None]
        rot_k = [None, None]
        with ExitStack() as actx:
            xt_pool = actx.enter_context(tc.tile_pool(name="xt", bufs=16))
            sq_pool = actx.enter_context(tc.tile_pool(name="sq", bufs=2))
            ab_pool = actx.enter_context(tc.tile_pool(name="ab", bufs=4))
            tmp_pool = actx.enter_context(tc.tile_pool(name="tmp", bufs=6))
            rs_pool = actx.enter_context(tc.tile_pool(name="rs", bufs=4))
            pq = actx.enter_context(tc.tile_pool(name="pq", bufs=2, space="PSUM"))
            pv = actx.enter_context(tc.tile_pool(name="pv", bufs=2, space="PSUM"))
            pssq = actx.enter_context(tc.tile_pool(name="pssq", bufs=2, space="PSUM"))
            psw = actx.enter_context(tc.tile_pool(name="psw", bufs=2, space="PSUM"))

            ar_q_in = dram.tile([1, S], F32, tag="arq")
            ar_q_out = dram.tile([1, S], F32, tag="arq")
            ar_k_in = dram.tile([1, S], F32, tag="ark")
            ar_k_out = dram.tile([1, S], F32, tag="ark")

            # ---- Q pass, then K pass (each: e-major projection + ssq) ----
            for wsb, bsb, gsb, dst, arin, arout in (
                (wq_sb, bq_sb, gq_sb, qT_sb, ar_q_in, ar_q_out),
                (wk_sb, bk_sb, gk_sb, kT_sb, ar_k_in, ar_k_out),
            ):
                if wsb is wk_sb:
                    nc.sync.dma_start(wk_all[:], wk[:].bitcast(F32R))
                for tcix in range(NTC):
                    tsl = slice(TC * tcix, TC * tcix + TC)
                    xt = [xt_pool.tile([128, TC], F32R, tag="xt", name=f"xt{i}")
                          for i in range(NDC)]
                    for dc in range(NDC):
                        nc.sync.dma_start(
                            xt[dc][:], xT[128 * dc:128 * dc + 128, tsl].bitcast(F32R)
                        )
                    ssq_ps = pssq.tile([1, TC], F32, tag="pssq")
                    for et in range(2):
                        esl = slice(128 * et, 128 * et + 128)
                        qp = pq.tile([128, TC], F32, tag="pq")
                        for dc in range(NDC):
                            nc.tensor.matmul(
                                qp[:], wsb[dc][:, esl], xt[dc][:],
                                start=(dc == 0), stop=(dc == NDC - 1),
                            )
                        nc.scalar.activation(
                            dst[et][:, tsl], qp[:], AF.Identity, bias=bsb[et][:]
                        )
                        sq = sq_pool.tile([128, TC], F32R, tag="sq")
                        nc.scalar.activation(sq[:], qp[:], AF.Square, bias=bsb[et][:])
                        nc.tensor.matmul(
                            ssq_ps[:], gsb[et][:], sq[:],
                            start=(et == 0), stop=(et == 1),
                        )
                    ssq_c = rs_pool.tile([1, TC], F32, tag="rs", name="ssq_c")
                    nc.vector.tensor_copy(ssq_c[:], ssq_ps[:])
                    nc.sync.dma_start(arin[0:1, tsl], ssq_c[:])
                nc.gpsimd.collective_compute(
                    "AllReduce", mybir.AluOpType.add, replica_groups=GROUPS,
                    ins=[arin[:].opt()], outs=[arout[:].opt()],
                )

            # ---- k RoPE (no AR dependency) emitted before V so DVE overlaps it ----
            for et in range(2):
                rot_q[et] = qk_pool.tile([128, S], BF16, tag="rot", name=f"rot_q{et}")
                rot_k[et] = qk_pool.tile([128, S], BF16, tag="rot", name=f"rot_k{et}")
            for et in range(2):
                _emit_rope(nc, tc, ab_pool, tmp_pool, psw, perm_sb, ra, rb, RSq,
                           kT_sb, rot_k, et, False)

            # ---- V pass (t-major) ----
            nc.sync.dma_start(wv_all[:], wv[:].bitcast(F32R))
            for tcix in range(NTC):
                tsl = slice(TC * tcix, TC * tcix + TC)
                xt = [xt_pool.tile([128, TC], F32R, tag="xt", name=f"xtv{i}")
                      for i in range(NDC)]
                for dc in range(NDC):
                    nc.sync.dma_start(
                        xt[dc][:], xT[128 * dc:128 * dc + 128, tsl].bitcast(F32R)
                    )
                for ts in range(4):
                    jt = 4 * tcix + ts
                    vsl = slice(128 * ts, 128 * ts + 128)
                    vp = pv.tile([128, EL], F32, tag="pv")
                    for dc in range(NDC):
                        nc.tensor.matmul(
                            vp[:], xt[dc][:, vsl], wv_sb[dc][:],
                            start=(dc == 0), stop=False,
                        )
                    nc.tensor.matmul(vp[:], ones1r[:], bv_sb[:], start=False, stop=True)
                    nc.scalar.activation(
                        v_sb[jt][:].rearrange("p (h c) -> p h c", c=65)[:, :, 0:64],
                        vp[:].rearrange("p (h c) -> p h c", c=64),
                        AF.Identity,
                    )

            # ---- rs_q (row-major broadcast) and rs_kT from the AR outputs ----
            for tcix in range(NTC):
                tsl = slice(TC * tcix, TC * tcix + TC)
                ssqf = rs_pool.tile([1, TC], F32, tag="rs", name="ssqf")
                nc.sync.dma_start(ssqf[:], ar_q_out[0:1, tsl])
                stdt = rs_pool.tile([1, TC], F32, tag="rs", name="stdt")
                nc.scalar.activation(stdt[:], ssqf[:], AF.Sqrt, scale=1.0 / D,
                                     bias=eps_t[0:1, :])
                rs = rs_pool.tile([1, TC], F32, tag="rs", name="rs")
                nc.vector.reciprocal(rs[:], stdt[:])
                nc.gpsimd.partition_broadcast(RSq[:, tsl], rs[0:1, :], channels=128)
            for et in range(2):
                _emit_rope(nc, tc, ab_pool, tmp_pool, psw, perm_sb, ra, rb, RSq,
                           qT_sb, rot_q, et, True)
            # rs_k row from AR output (contiguous DMA), then transpose each
            # 128-token block to a [128,1] column via a K=1 matmul against ones.
            ssqk = small.tile([1, S], F32, tag="ssqk")
            nc.sync.dma_start(ssqk[:], ar_k_out[0:1, :])
            stdk = small.tile([1, S], F32, tag="stdk")
            nc.scalar.activation(stdk[:], ssqk[:], AF.Sqrt, scale=1.0 / D,
                                 bias=eps_t[0:1, :])
            rskrow = small.tile([1, S], F32, tag="rskrow")
            nc.vector.reciprocal(rskrow[:], stdk[:])
            for jc in range(NJC):
                cps = pssq.tile([128, 1], F32, tag="pssq", name="cps")
                nc.tensor.matmul(
                    cps[:], rskrow[0:1, 128 * jc:128 * jc + 128],
                    ones1[0:1, 0:1], start=True, stop=True,
                )
                nc.scalar.mul(rskT[:, jc:jc + 1], cps[:], 1.0 / float(np.sqrt(DH)))


        # ---------------- phase B: attention + AG + out-proj ----------------
        with ExitStack() as bctx:
            exp_pool = bctx.enter_context(tc.tile_pool(name="exp", bufs=3))
            og_pool = bctx.enter_context(tc.tile_pool(name="og", bufs=8))
            dv_pool = bctx.enter_context(tc.tile_pool(name="dv", bufs=4))
            y_pool = bctx.enter_context(tc.tile_pool(name="y", bufs=2))
            rec_pool = bctx.enter_context(tc.tile_pool(name="rec", bufs=2))
            psc = bctx.enter_context(tc.tile_pool(name="psc", bufs=2, space="PSUM"))
            pav = bctx.enter_context(tc.tile_pool(name="pav", bufs=3, space="PSUM"))
            py = bctx.enter_context(tc.tile_pool(name="py", bufs=1, space="PSUM"))

            wo_all = w_pool.tile([128, NDC * EL], BF16, tag="wo", name="wo_all",
                                 bufs=1)
            wo_sb = [wo_all[:, EL * i:EL * i + EL] for i in range(NDC)]
            nc.gpsimd.dma_start(wo_all[:], wo[:])

            for itix in range(NTC):           # i-tile loop (queries)
                isl = slice(TC * itix, TC * itix + TC)
                ag_in = dram.tile([EL, TC], BF16, tag="agin")
                for hp in range(2):           # head pair
                    q_t, k_t = rot_q[hp], rot_k[hp]
                    av = [pav.tile([65, TC], F32, tag="pav", name=f"av{i}") for i in range(2)]
                    for jc in range(NJC):
                        jsl = slice(128 * jc, 128 * jc + 128)
                        sc = psc.tile([128, 2 * TC], F32, tag="psc")
                        for hh in range(2):
                            psl = slice(64 * hh, 64 * hh + 64)
                            nc.tensor.matmul(
                                sc[:, TC * hh:TC * hh + TC],
                                k_t[psl, jsl], q_t[psl, isl],
                                start=True, stop=True,
                            )
                        ex = exp_pool.tile([128, 2 * TC], BF16, tag="exp")
                        nc.scalar.activation(
                            ex[:], sc[:], AF.Exp, scale=rskT[:, jc:jc + 1]
                        )
                        for hh in range(2):
                            h4 = 2 * hp + hh
                            nc.tensor.matmul(
                                av[hh][:],
                                v_sb[jc][:, 65 * h4:65 * h4 + 65],
                                ex[:, TC * hh:TC * hh + TC],
                                start=(jc == 0), stop=(jc == NJC - 1),
                            )
                    for hh in range(2):
                        h4 = 2 * hp + hh
                        avs = dv_pool.tile([65, TC], F32, tag="avs")
                        nc.vector.tensor_copy(avs[:], av[hh][:])
                        rec = rec_pool.tile([1, TC], F32, tag="rec")
                        nc.vector.reciprocal(rec[:], avs[64:65, :])
                        rcb = rec_pool.tile([64, TC], F32, tag="recb")
                        nc.gpsimd.partition_broadcast(rcb[:], rec[0:1, :], channels=64)
                        dv = dv_pool.tile([64, TC], BF16, tag="dv")
                        nc.vector.tensor_tensor(dv[:], avs[0:64, :], rcb[:], MUL)
                        nc.sync.dma_start(ag_in[64 * h4:64 * h4 + 64, :], dv[:])
                # AllGather this i-tile's outputs across the batch group
                ag_out = dram.tile([4 * EL, TC], BF16, tag="agout")
                nc.gpsimd.collective_compute(
                    "AllGather", mybir.AluOpType.bypass, replica_groups=GROUPS,
                    ins=[ag_in[:].opt()], outs=[ag_out[:].opt()],
                )
                og = [og_pool.tile([128, TC], BF16, tag="og", name=f"og{i}") for i in range(NDC)]
                for ec in range(NDC):
                    nc.sync.dma_start(og[ec][:], ag_out[128 * ec:128 * ec + 128, :])
                for dt2 in range(2):
                    dsl = slice(128 * dt2, 128 * dt2 + 128)
                    yp = py.tile([128, TC], F32, tag="py")
                    for ec in range(NDC):
                        nc.tensor.matmul(
                            yp[:], wo_sb[ec][:, dsl], og[ec][:],
                            start=(ec == 0), stop=(ec == NDC - 1),
                        )
                    ys = y_pool.tile([128, TC], F32, tag="y")
                    nc.scalar.activation(ys[:], yp[:], AF.Identity, bias=bo_sb[dt2][:])
                    nc.sync.dma_start(yT[dsl, isl], ys[:])


def build_nc():
    nc = bacc.Bacc("TRN2", target_bir_lowering=False, debug=False, num_devices=N_CORES)
    _emit(nc)
    nc.compile()
    return nc


_NC_CACHE = None


def _get_nc():
    global _NC_CACHE
    if _NC_CACHE is None:
        _NC_CACHE = build_nc()
    return _NC_CACHE


def _host_prep(inputs):
    x = np.ascontiguousarray(np.asarray(inputs["x"], dtype=np.float32))
    pe = np.asarray(inputs["pe"], dtype=np.float32)[0, 0]      # [S, D//2, 2, 2]
    Wq = np.asarray(inputs["Wq"], dtype=np.float32)
    bq = np.asarray(inputs["bq"], dtype=np.float32)
    Wk = np.asarray(inputs["Wk"], dtype=np.float32)
    bk = np.asarray(inputs["bk"], dtype=np.float32)
    Wv = np.asarray(inputs["Wv"], dtype=np.float32)
    bv = np.asarray(inputs["bv"], dtype=np.float32)
    qn = np.asarray(inputs["qn_w"], dtype=np.float32)
    kn = np.asarray(inputs["kn_w"], dtype=np.float32)
    Wo = np.asarray(inputs["Wo"], dtype=np.float32)
    bo = np.asarray(inputs["bo"], dtype=np.float32)

    Wq_f = Wq * qn[:, None]
    bq_f = qn * bq
    Wk_f = Wk * kn[:, None]
    bk_f = kn * bk

    A = np.empty((D, S), np.float32)
    Bm = np.empty((D, S), np.float32)
    A[0::2, :] = pe[:, :, 0, 0].T
    A[1::2, :] = pe[:, :, 1, 1].T
    Bm[0::2, :] = pe[:, :, 0, 1].T
    Bm[1::2, :] = pe[:, :, 1, 0].T

    perm = np.zeros((128, 128), np.float32)
    idx = np.arange(64)
    perm[2 * idx, 2 * idx + 1] = 1.0
    perm[2 * idx + 1, 2 * idx] = 1.0

    def pack_dT(m):          # m: [D, F] -> [128, NDC*F] (dc-major wide rows)
        F = m.shape[1]
        return np.ascontiguousarray(
            m.reshape(NDC, 128, F).transpose(1, 0, 2).reshape(128, NDC * F)
        )
    xTs = [np.ascontiguousarray(x[b].T) for b in range(B)]
    in_maps = []
    for c in range(N_CORES):
        b, g = c // 4, c % 4
        E = slice(256 * g, 256 * g + 256)
        in_maps.append({
            "xT": xTs[b],
            "wq": pack_dT(Wq_f[E, :].T.copy()),
            "wk": pack_dT(Wk_f[E, :].T.copy()),
            "wv": pack_dT(Wv[E, :].T.copy()),
            "wo": pack_dT(Wo[E, :].T.copy()),
            "bq": np.ascontiguousarray(bq_f[E])[:, None],
            "bk": np.ascontiguousarray(bk_f[E])[:, None],
            "bv": np.ascontiguousarray(bv[E])[None, :],
            "bo": np.ascontiguousarray(bo[E])[:, None],
            "gq": np.ascontiguousarray(1.0 / qn[E] ** 2)[:, None],
            "gk": np.ascontiguousarray(1.0 / kn[E] ** 2)[:, None],
            "rope_a": np.ascontiguousarray(A[E, :]),
            "rope_b": np.ascontiguousarray(Bm[E, :]),
            "perm": perm,
        })
    return in_maps


def kernel(**inputs):
    nc = _get_nc()
    in_maps = _host_prep(inputs)
    res = run_bass_kernel_spmd(
        nc, in_maps, core_ids=list(range(N_CORES)), trace=TRACE
    )
    if TRACE and res.exec_time_ns is not None:
        print(f"HW exec time: {res.exec_time_ns} ns")
    y = np.empty((B, S, D), np.float32)
    for c in range(N_CORES):
        b, g = c // 4, c % 4
        y[b][:, 256 * g:256 * g + 256] = res.results[c]["yT"].T
    return y



# revision 4
# speedup vs baseline: 1.1362x; 1.1362x over previous
"""Distributed CrossAttention (self-attention) kernel for 8 TRN2 NeuronCores.

Problem: B=2, S=2048, D=1024, H=16, DH=64, fp32.
  q/k/v = x@W.T + b; RMSNorm(q/k over full D); RoPE; SDPA; out-proj.

Sharding: core c -> (batch b = c//4, head-group g = c%4) = 4 heads = 256 dims.
Everything on-device is feature-major ("transposed"): the host pre-transposes x
and the weight slices (packed into wide contiguous rows for DMA descriptor
efficiency), so no on-device transposes are needed.

Per-core pipeline (v2):
  0. x is DMA'd ONCE in bf16 ([128,S] per 128-dim chunk) and kept in SBUF for
     all three projection passes; weights are bf16. A tiny warm-up AllReduce
     is issued at t=0 so the CC engine's cold-start cost burns under the
     projection matmuls.
  1. K then Q e-major projections ([256,S] bf16 matmuls, W-stationary) with
     qn_w/kn_w folded into the weights on the host; per-token sum-of-squares
     of the raw k/q via a g=1/w^2 column matmul; ONE combined AllReduce over
     the [2,S] (k;q) ssq rows per 4-core batch group.
  2. k RoPE (no AR dependency) + V projection t-major (x-stationary) into a
     padded [t, 4*(64+1)] layout whose 65th column is ones - the softmax
     denominator rides along the AV matmul for free. These overlap the AR.
  3. RoPE as A*q + B*(P@q): pair-swap permutation matmul + host-prepared
     e-major pe coefficient planes A/B; rs_q = 1/sqrt(mean(q^2)+eps) folded
     in via gpsimd partition_broadcast; reciprocals via the fast DVE approx.
  4. SDPA per head pair in bf16: scoresT = k_h.T @ q_h with K=64 PE row-tiling
     (partition bases 0/64); exp on ScalarE straight out of PSUM [128,1024]
     with rs_k/sqrt(DH) as its per-partition scale operand; AV accumulates
     over j-chunks; divide via fast reciprocal + gpsimd broadcast.
  5. Per-i-tile bf16 AllGather of attention output over the batch group; the
     out-projection for i-tile N-1 is emitted AFTER attention for i-tile N so
     the AllGather latency hides under attention instead of stalling the
     in-order PE queue.
"""
import numpy as np
from contextlib import ExitStack

import concourse.bass as bass
import concourse.mybir as mybir
import concourse.tile as tile
import concourse.bacc as bacc
from concourse.bass_utils import run_bass_kernel_spmd

F32 = mybir.dt.float32
F32R = mybir.dt.float32r
BF16 = mybir.dt.bfloat16
AF = mybir.ActivationFunctionType
MUL = mybir.AluOpType.mult

B, S, D, H, DH = 2, 2048, 1024, 16, 64
EPS = 1e-5
N_CORES = 8
EL = 256            # e-dims per core
NHL = 4             # heads per core
TC = 512            # token chunk (matmul moving N)
NTC = S // TC       # 4
NDC = D // 128      # 8 contraction chunks
NJC = S // 128      # 16 key chunks
GROUPS = [[0, 1, 2, 3], [4, 5, 6, 7]]

TRACE = False       # test.py flips this for profiling


def _emit_rope(nc, tc, ab_pool, tmp_pool, psw, perm_sb, ra, rb, RSq, srct, rot, et, is_q):
    esl = slice(128 * et, 128 * et + 128)
    for tcix in range(NTC):
        tsl = slice(TC * tcix, TC * tcix + TC)
        at = ab_pool.tile([128, TC], F32, tag="ab", name="at")
        bt = ab_pool.tile([128, TC], F32, tag="ab", name="bt")
        nc.sync.dma_start(at[:], ra[esl, tsl])
        nc.sync.dma_start(bt[:], rb[esl, tsl])
        swp = psw.tile([128, TC], F32, tag="psw")
        nc.tensor.matmul(swp[:], perm_sb[:], srct[et][:, tsl], start=True, stop=True)
        t1 = tmp_pool.tile([128, TC], F32, tag="tmp")
        nc.vector.tensor_tensor(t1[:], at[:], srct[et][:, tsl], MUL)
        t2 = tmp_pool.tile([128, TC], F32, tag="tmp")
        nc.vector.tensor_tensor(t2[:], bt[:], swp[:], MUL)
        if is_q:
            t3 = tmp_pool.tile([128, TC], F32, tag="tmp")
            nc.vector.tensor_add(t3[:], t1[:], t2[:])
            nc.vector.tensor_tensor(rot[et][:, tsl], t3[:], RSq[:, tsl], MUL)
        else:
            nc.vector.tensor_add(rot[et][:, tsl], t1[:], t2[:])


def _emit(nc):
    xT = nc.declare_dram_parameter("xT", [D, S], BF16, isOutput=False)
    wq = nc.declare_dram_parameter("wq", [128, NDC * EL], BF16, isOutput=False)
    wk = nc.declare_dram_parameter("wk", [128, NDC * EL], BF16, isOutput=False)
    wv = nc.declare_dram_parameter("wv", [128, NDC * EL], BF16, isOutput=False)
    wo = nc.declare_dram_parameter("wo", [128, NDC * EL], BF16, isOutput=False)
    bq = nc.declare_dram_parameter("bq", [EL, 1], F32, isOutput=False)
    bk = nc.declare_dram_parameter("bk", [EL, 1], F32, isOutput=False)
    bv = nc.declare_dram_parameter("bv", [1, EL], BF16, isOutput=False)
    bo = nc.declare_dram_parameter("bo", [EL, 1], F32, isOutput=False)
    gq = nc.declare_dram_parameter("gq", [EL, 1], F32, isOutput=False)
    gk = nc.declare_dram_parameter("gk", [EL, 1], F32, isOutput=False)
    ra = nc.declare_dram_parameter("rope_a", [EL, S], F32, isOutput=False)
    rb = nc.declare_dram_parameter("rope_b", [EL, S], F32, isOutput=False)
    pm = nc.declare_dram_parameter("perm", [128, 128], F32, isOutput=False)
    yT = nc.declare_dram_parameter("yT", [EL, S], F32, isOutput=True)

    with tile.TileContext(nc) as tc, ExitStack() as ctx:
        # ---------------- persistent pools ----------------
        w_pool = ctx.enter_context(tc.tile_pool(name="w", bufs=1))
        x_pool = ctx.enter_context(tc.tile_pool(name="x", bufs=1))
        qk_pool = ctx.enter_context(tc.tile_pool(name="qk", bufs=4))
        v_pool = ctx.enter_context(tc.tile_pool(name="v", bufs=16))
        small = ctx.enter_context(tc.tile_pool(name="small", bufs=1))
        rsq_pool = ctx.enter_context(tc.tile_pool(name="rsq", bufs=1))
        dram = ctx.enter_context(tc.tile_pool(name="dram", bufs=16, space="DRAM"))

        # ---------------- CC warm-up: tiny AllReduce at t=0 ----------------
        warm_in = dram.tile([1, 8], F32, tag="warm")
        warm_out = dram.tile([1, 8], F32, tag="warm")
        wz = small.tile([1, 8], F32, tag="wz")
        nc.vector.memset(wz[:], 0.0)
        nc.sync.dma_start(warm_in[:], wz[:])
        nc.gpsimd.collective_compute(
            "AllReduce", mybir.AluOpType.add, replica_groups=GROUPS,
            ins=[warm_in[:].opt()], outs=[warm_out[:].opt()],
        )

        # ---------------- constants / small loads ----------------
        bq_sb, bk_sb, bo_sb, gq_sb, gk_sb = [], [], [], [], []
        for et in range(2):
            sl = slice(128 * et, 128 * et + 128)
            t = small.tile([128, 1], F32, tag=f"bq{et}", name=f"bq{et}")
            nc.sync.dma_start(t[:], bq[sl, :]); bq_sb.append(t)
            t = small.tile([128, 1], F32, tag=f"bk{et}", name=f"bk{et}")
            nc.sync.dma_start(t[:], bk[sl, :]); bk_sb.append(t)
            t = small.tile([128, 1], F32, tag=f"bo{et}", name=f"bo{et}")
            nc.sync.dma_start(t[:], bo[sl, :]); bo_sb.append(t)
            t = small.tile([128, 1], F32R, tag=f"gq{et}", name=f"gq{et}")
            nc.sync.dma_start(t[:], gq[sl, :].bitcast(F32R)); gq_sb.append(t)
            t = small.tile([128, 1], F32R, tag=f"gk{et}", name=f"gk{et}")
            nc.sync.dma_start(t[:], gk[sl, :].bitcast(F32R)); gk_sb.append(t)
        bv_sb = small.tile([1, EL], BF16, tag="bvrow")
        nc.sync.dma_start(bv_sb[:], bv[:])
        ones1 = small.tile([1, 128], F32, tag="ones1")
        nc.vector.memset(ones1[:], 1.0)
        ones1b = small.tile([1, 128], BF16, tag="ones1b")
        nc.vector.tensor_copy(ones1b[:], ones1[:])
        ones4 = small.tile([128, 4], F32, tag="ones4")
        nc.vector.memset(ones4[:], 1.0)
        eps_t = small.tile([2, 1], F32, tag="eps")
        nc.vector.memset(eps_t[:], EPS)
        perm_sb = small.tile([128, 128], F32R, tag="perm")
        nc.sync.dma_start(perm_sb[:], pm[:].bitcast(F32R))

        # ---------------- weights (bf16) + x (bf16, loaded ONCE) ----------
        wk_all = w_pool.tile([128, NDC * EL], BF16, tag="wk", name="wk_all", bufs=1)
        wq_all = w_pool.tile([128, NDC * EL], BF16, tag="wq", name="wq_all", bufs=1)
        wv_all = w_pool.tile([128, NDC * EL], BF16, tag="wv", name="wv_all", bufs=1)
        wo_all = w_pool.tile([128, NDC * EL], BF16, tag="wo", name="wo_all", bufs=1)
        wq_sb = [wq_all[:, EL * i:EL * i + EL] for i in range(NDC)]
        wk_sb = [wk_all[:, EL * i:EL * i + EL] for i in range(NDC)]
        wv_sb = [wv_all[:, EL * i:EL * i + EL] for i in range(NDC)]
        wo_sb = [wo_all[:, EL * i:EL * i + EL] for i in range(NDC)]
        nc.sync.dma_start(wk_all[:], wk[:])
        nc.sync.dma_start(wq_all[:], wq[:])
        nc.sync.dma_start(wv_all[:], wv[:])
        nc.sync.dma_start(wo_all[:], wo[:])

        xt = [x_pool.tile([128, S], BF16, tag=f"x{i}", name=f"xt{i}", bufs=1)
              for i in range(NDC)]
        for dc in range(NDC):
            nc.sync.dma_start(xt[dc][:], xT[128 * dc:128 * dc + 128, :])

        # persistent activation tiles
        qT_sb = [qk_pool.tile([128, S], F32R, tag="qk", name=f"qT_sb{i}") for i in range(2)]
        kT_sb = [qk_pool.tile([128, S], F32R, tag="qk", name=f"kT_sb{i}") for i in range(2)]
        v_sb = [v_pool.tile([128, NHL * 65], BF16, tag="v", name=f"v_sb{i}") for i in range(NJC)]
        for jt in range(NJC):
            ones_cols = v_sb[jt][:].rearrange("p (h c) -> p h c", c=65)[:, :, 64:65]
            nc.vector.tensor_copy(
                ones_cols, ones4[:].rearrange("p (f o) -> p f o", o=1)
            )
        RSq = rsq_pool.tile([128, S], F32, tag="rsq")
        rskT = small.tile([128, NJC], F32, tag="rskT")

        # ---------------- phase A: projections + ssq + rope ----------------
        rot_q = [None, None]
        rot_k = [None, None]
        with ExitStack() as actx:
            sq_pool = actx.enter_context(tc.tile_pool(name="sq", bufs=2))
            ab_pool = actx.enter_context(tc.tile_pool(name="ab", bufs=4))
            tmp_pool = actx.enter_context(tc.tile_pool(name="tmp", bufs=6))
            rs_pool = actx.enter_context(tc.tile_pool(name="rs", bufs=4))
            pq = actx.enter_context(tc.tile_pool(name="pq", bufs=2, space="PSUM"))
            pv = actx.enter_context(tc.tile_pool(name="pv", bufs=2, space="PSUM"))
            pssq = actx.enter_context(tc.tile_pool(name="pssq", bufs=2, space="PSUM"))
            psw = actx.enter_context(tc.tile_pool(name="psw", bufs=2, space="PSUM"))

            ar_in = dram.tile([2, S], F32, tag="arin")
            ar_out = dram.tile([2, S], F32, tag="arout")

            # ---- K pass, then Q pass (each: e-major projection + ssq) ----
            # k ssq -> ar row 0, q ssq -> ar row 1; ONE combined AllReduce.
            for wsb, bsb, gsb, dst, arrow in (
                (wk_sb, bk_sb, gk_sb, kT_sb, 0),
                (wq_sb, bq_sb, gq_sb, qT_sb, 1),
            ):
                for tcix in range(NTC):
                    tsl = slice(TC * tcix, TC * tcix + TC)
                    ssq_ps = pssq.tile([1, TC], F32, tag="pssq")
                    for et in range(2):
                        esl = slice(128 * et, 128 * et + 128)
                        qp = pq.tile([128, TC], F32, tag="pq")
                        for dc in range(NDC):
                            nc.tensor.matmul(
                                qp[:], wsb[dc][:, esl], xt[dc][:, tsl],
                                start=(dc == 0), stop=(dc == NDC - 1),
                            )
                        nc.scalar.activation(
                            dst[et][:, tsl], qp[:], AF.Identity, bias=bsb[et][:]
                        )
                        sq = sq_pool.tile([128, TC], F32R, tag="sq")
                        nc.scalar.activation(sq[:], qp[:], AF.Square, bias=bsb[et][:])
                        nc.tensor.matmul(
                            ssq_ps[:], gsb[et][:], sq[:],
                            start=(et == 0), stop=(et == 1),
                        )
                    ssq_c = rs_pool.tile([1, TC], F32, tag="rs", name="ssq_c")
                    nc.vector.tensor_copy(ssq_c[:], ssq_ps[:])
                    nc.sync.dma_start(ar_in[arrow:arrow + 1, tsl], ssq_c[:])
            nc.gpsimd.collective_compute(
                "AllReduce", mybir.AluOpType.add, replica_groups=GROUPS,
                ins=[ar_in[:].opt()], outs=[ar_out[:].opt()],
            )

            # ---- k RoPE (no AR dependency) emitted before V so DVE overlaps it ----
            for et in range(2):
                rot_q[et] = qk_pool.tile([128, S], BF16, tag="rot", name=f"rot_q{et}")
                rot_k[et] = qk_pool.tile([128, S], BF16, tag="rot", name=f"rot_k{et}")
            for et in range(2):
                _emit_rope(nc, tc, ab_pool, tmp_pool, psw, perm_sb, ra, rb, RSq,
                           kT_sb, rot_k, et, False)

            # ---- V pass (t-major) ----
            for tcix in range(NTC):
                for ts in range(4):
                    jt = 4 * tcix + ts
                    vsl = slice(TC * tcix + 128 * ts, TC * tcix + 128 * ts + 128)
                    vp = pv.tile([128, EL], F32, tag="pv")
                    for dc in range(NDC):
                        nc.tensor.matmul(
                            vp[:], xt[dc][:, vsl], wv_sb[dc][:],
                            start=(dc == 0), stop=False,
                        )
                    nc.tensor.matmul(vp[:], ones1b[:], bv_sb[:], start=False, stop=True)
                    nc.scalar.activation(
                        v_sb[jt][:].rearrange("p (h c) -> p h c", c=65)[:, :, 0:64],
                        vp[:].rearrange("p (h c) -> p h c", c=64),
                        AF.Identity,
                    )

            # ---- rs_q (row-major broadcast) from AR row 1 ----
            ssqq = small.tile([1, S], F32, tag="ssqq")
            nc.sync.dma_start(ssqq[:], ar_out[1:2, :])
            stdq = small.tile([1, S], F32, tag="stdq")
            nc.scalar.activation(stdq[:], ssqq[:], AF.Sqrt, scale=1.0 / D,
                                 bias=eps_t[0:1, :])
            rsqrow = small.tile([1, S], F32, tag="rsqrow")
            nc.vector.reciprocal(rsqrow[:], stdq[:])
            for tcix in range(NTC):
                tsl = slice(TC * tcix, TC * tcix + TC)
                nc.gpsimd.partition_broadcast(RSq[:, tsl], rsqrow[0:1, tsl],
                                              channels=128)
            for et in range(2):
                _emit_rope(nc, tc, ab_pool, tmp_pool, psw, perm_sb, ra, rb, RSq,
                           qT_sb, rot_q, et, True)
            # rs_k row from AR row 0 (contiguous DMA), then transpose each
            # 128-token block to a [128,1] column via a K=1 matmul against ones.
            ssqk = small.tile([1, S], F32, tag="ssqk")
            nc.sync.dma_start(ssqk[:], ar_out[0:1, :])
            stdk = small.tile([1, S], F32, tag="stdk")
            nc.scalar.activation(stdk[:], ssqk[:], AF.Sqrt, scale=1.0 / D,
                                 bias=eps_t[0:1, :])
            rskrow = small.tile([1, S], F32, tag="rskrow")
            nc.vector.reciprocal(rskrow[:], stdk[:])
            for jc in range(NJC):
                cps = pssq.tile([128, 1], F32, tag="pssq", name="cps")
                nc.tensor.matmul(
                    cps[:], rskrow[0:1, 128 * jc:128 * jc + 128],
                    ones1[0:1, 0:1], start=True, stop=True,
                )
                nc.scalar.mul(rskT[:, jc:jc + 1], cps[:], 1.0 / float(np.sqrt(DH)))

        # ---------------- phase B: attention + AG + out-proj ----------------
        with ExitStack() as bctx:
            exp_pool = bctx.enter_context(tc.tile_pool(name="exp", bufs=3))
            og_pool = bctx.enter_context(tc.tile_pool(name="og", bufs=8))
            dv_pool = bctx.enter_context(tc.tile_pool(name="dv", bufs=4))
            y_pool = bctx.enter_context(tc.tile_pool(name="y", bufs=2))
            rec_pool = bctx.enter_context(tc.tile_pool(name="rec", bufs=2))
            psc = bctx.enter_context(tc.tile_pool(name="psc", bufs=2, space="PSUM"))
            pav = bctx.enter_context(tc.tile_pool(name="pav", bufs=3, space="PSUM"))
            py = bctx.enter_context(tc.tile_pool(name="py", bufs=1, space="PSUM"))

            def emit_outproj(ag_out, isl):
                og = [og_pool.tile([128, TC], BF16, tag="og", name=f"og{i}")
                      for i in range(NDC)]
                for ec in range(NDC):
                    nc.sync.dma_start(og[ec][:], ag_out[128 * ec:128 * ec + 128, :])
                for dt2 in range(2):
                    dsl = slice(128 * dt2, 128 * dt2 + 128)
                    yp = py.tile([128, TC], F32, tag="py")
                    for ec in range(NDC):
                        nc.tensor.matmul(
                            yp[:], wo_sb[ec][:, dsl], og[ec][:],
                            start=(ec == 0), stop=(ec == NDC - 1),
                        )
                    ys = y_pool.tile([128, TC], F32, tag="y")
                    nc.scalar.activation(ys[:], yp[:], AF.Identity, bias=bo_sb[dt2][:])
                    nc.sync.dma_start(yT[dsl, isl], ys[:])

            prev = None                       # (ag_out, isl) of the previous i-tile
            for itix in range(NTC):           # i-tile loop (queries)
                isl = slice(TC * itix, TC * itix + TC)
                ag_in = dram.tile([EL, TC], BF16, tag="agin")
                for hp in range(2):           # head pair
                    q_t, k_t = rot_q[hp], rot_k[hp]
                    av = [pav.tile([65, TC], F32, tag="pav", name=f"av{i}") for i in range(2)]
                    for jc in range(NJC):
                        jsl = slice(128 * jc, 128 * jc + 128)
                        sc = psc.tile([128, 2 * TC], F32, tag="psc")
                        for hh in range(2):
                            psl = slice(64 * hh, 64 * hh + 64)
                            nc.tensor.matmul(
                                sc[:, TC * hh:TC * hh + TC],
                                k_t[psl, jsl], q_t[psl, isl],
                                start=True, stop=True,
                            )
                        ex = exp_pool.tile([128, 2 * TC], BF16, tag="exp")
                        nc.scalar.activation(
                            ex[:], sc[:], AF.Exp, scale=rskT[:, jc:jc + 1]
                        )
                        for hh in range(2):
                            h4 = 2 * hp + hh
                            nc.tensor.matmul(
                                av[hh][:],
                                v_sb[jc][:, 65 * h4:65 * h4 + 65],
                                ex[:, TC * hh:TC * hh + TC],
                                start=(jc == 0), stop=(jc == NJC - 1),
                            )
                    for hh in range(2):
                        h4 = 2 * hp + hh
                        avs = dv_pool.tile([65, TC], F32, tag="avs")
                        nc.vector.tensor_copy(avs[:], av[hh][:])
                        rec = rec_pool.tile([1, TC], F32, tag="rec")
                        nc.vector.reciprocal(rec[:], avs[64:65, :])
                        rcb = rec_pool.tile([64, TC], F32, tag="recb")
                        nc.gpsimd.partition_broadcast(rcb[:], rec[0:1, :], channels=64)
                        dv = dv_pool.tile([64, TC], BF16, tag="dv")
                        nc.vector.tensor_tensor(dv[:], avs[0:64, :], rcb[:], MUL)
                        nc.sync.dma_start(ag_in[64 * h4:64 * h4 + 64, :], dv[:])
                # AllGather this i-tile's outputs across the batch group
                ag_out = dram.tile([4 * EL, TC], BF16, tag="agout")
                nc.gpsimd.collective_compute(
                    "AllGather", mybir.AluOpType.bypass, replica_groups=GROUPS,
                    ins=[ag_in[:].opt()], outs=[ag_out[:].opt()],
                )
                # out-proj for the PREVIOUS i-tile: its AllGather completed
                # while this i-tile's attention was running, so the PE never
                # waits on a collective.
                if prev is not None:
                    emit_outproj(*prev)
                prev = (ag_out, isl)
            emit_outproj(*prev)


def build_nc():
    nc = bacc.Bacc("TRN2", target_bir_lowering=False, debug=False, num_devices=N_CORES)
    _emit(nc)
    nc.compile()
    return nc


_NC_CACHE = None


def _get_nc():
    global _NC_CACHE
    if _NC_CACHE is None:
        _NC_CACHE = build_nc()
    return _NC_CACHE


def _host_prep(inputs):
    import ml_dtypes
    bf16 = ml_dtypes.bfloat16

    x = np.ascontiguousarray(np.asarray(inputs["x"], dtype=np.float32))
    pe = np.asarray(inputs["pe"], dtype=np.float32)[0, 0]      # [S, D//2, 2, 2]
    Wq = np.asarray(inputs["Wq"], dtype=np.float32)
    bq = np.asarray(inputs["bq"], dtype=np.float32)
    Wk = np.asarray(inputs["Wk"], dtype=np.float32)
    bk = np.asarray(inputs["bk"], dtype=np.float32)
    Wv = np.asarray(inputs["Wv"], dtype=np.float32)
    bv = np.asarray(inputs["bv"], dtype=np.float32)
    qn = np.asarray(inputs["qn_w"], dtype=np.float32)
    kn = np.asarray(inputs["kn_w"], dtype=np.float32)
    Wo = np.asarray(inputs["Wo"], dtype=np.float32)
    bo = np.asarray(inputs["bo"], dtype=np.float32)

    Wq_f = Wq * qn[:, None]
    bq_f = qn * bq
    Wk_f = Wk * kn[:, None]
    bk_f = kn * bk

    A = np.empty((D, S), np.float32)
    Bm = np.empty((D, S), np.float32)
    A[0::2, :] = pe[:, :, 0, 0].T
    A[1::2, :] = pe[:, :, 1, 1].T
    Bm[0::2, :] = pe[:, :, 0, 1].T
    Bm[1::2, :] = pe[:, :, 1, 0].T

    perm = np.zeros((128, 128), np.float32)
    idx = np.arange(64)
    perm[2 * idx, 2 * idx + 1] = 1.0
    perm[2 * idx + 1, 2 * idx] = 1.0

    def pack_dT(m):          # m: [D, F] -> [128, NDC*F] (dc-major wide rows)
        F = m.shape[1]
        return np.ascontiguousarray(
            m.reshape(NDC, 128, F).transpose(1, 0, 2).reshape(128, NDC * F)
        ).astype(bf16)
    xTs = [np.ascontiguousarray(x[b].T).astype(bf16) for b in range(B)]
    in_maps = []
    for c in range(N_CORES):
        b, g = c // 4, c % 4
        E = slice(256 * g, 256 * g + 256)
        in_maps.append({
            "xT": xTs[b],
            "wq": pack_dT(Wq_f[E, :].T.copy()),
            "wk": pack_dT(Wk_f[E, :].T.copy()),
            "wv": pack_dT(Wv[E, :].T.copy()),
            "wo": pack_dT(Wo[E, :].T.copy()),
            "bq": np.ascontiguousarray(bq_f[E])[:, None],
            "bk": np.ascontiguousarray(bk_f[E])[:, None],
            "bv": np.ascontiguousarray(bv[E])[None, :].astype(bf16),
            "bo": np.ascontiguousarray(bo[E])[:, None],
            "gq": np.ascontiguousarray(1.0 / qn[E] ** 2)[:, None],
            "gk": np.ascontiguousarray(1.0 / kn[E] ** 2)[:, None],
            "rope_a": np.ascontiguousarray(A[E, :]),
            "rope_b": np.ascontiguousarray(Bm[E, :]),
            "perm": perm,
        })
    return in_maps


def kernel(**inputs):
    nc = _get_nc()
    in_maps = _host_prep(inputs)
    res = run_bass_kernel_spmd(
        nc, in_maps, core_ids=list(range(N_CORES)), trace=TRACE
    )
    if TRACE and res.exec_time_ns is not None:
        print(f"HW exec time: {res.exec_time_ns} ns")
    y = np.empty((B, S, D), np.float32)
    for c in range(N_CORES):
        b, g = c // 4, c % 4
        y[b][:, 256 * g:256 * g + 256] = res.results[c]["yT"].T
    return y


# revision 7
# speedup vs baseline: 1.2873x; 1.1329x over previous
"""Distributed CrossAttention (self-attention) kernel for 8 TRN2 NeuronCores.

Problem: B=2, S=2048, D=1024, H=16, DH=64, fp32.
  q/k/v = x@W.T + b; RMSNorm(q/k over full D); RoPE; SDPA; out-proj.

Sharding: core c -> (batch b = c//4, head-group g = c%4) = 4 heads = 256 dims.
Everything on-device is feature-major ("transposed"): the host pre-transposes x
and the weight slices (packed into wide contiguous rows for DMA descriptor
efficiency), so no on-device transposes are needed.

Per-core pipeline (v2):
  0. x is DMA'd ONCE in bf16 ([128,S] per 128-dim chunk) and kept in SBUF for
     all three projection passes; weights are bf16. A tiny warm-up AllReduce
     is issued at t=0 so the CC engine's cold-start cost burns under the
     projection matmuls.
  1. K then Q e-major projections ([256,S] bf16 matmuls, W-stationary) with
     qn_w/kn_w folded into the weights on the host; per-token sum-of-squares
     of the raw k/q via a g=1/w^2 column matmul; ONE combined AllReduce over
     the [2,S] (k;q) ssq rows per 4-core batch group.
  2. k RoPE (no AR dependency) + V projection t-major (x-stationary) into a
     padded [t, 4*(64+1)] layout whose 65th column is ones - the softmax
     denominator rides along the AV matmul for free. These overlap the AR.
  3. RoPE as A*q + B*(P@q): pair-swap permutation matmul + host-prepared
     e-major pe coefficient planes A/B; rs_q = 1/sqrt(mean(q^2)+eps) folded
     in via gpsimd partition_broadcast; reciprocals via the fast DVE approx.
  4. SDPA per head pair in bf16: scoresT = k_h.T @ q_h with K=64 PE row-tiling
     (partition bases 0/64); exp on ScalarE straight out of PSUM [128,1024]
     with rs_k/sqrt(DH) as its per-partition scale operand; AV accumulates
     over j-chunks; divide via fast reciprocal + gpsimd broadcast.
  5. Per-i-tile bf16 AllGather of attention output over the batch group; the
     out-projection for i-tile N-1 is emitted AFTER attention for i-tile N so
     the AllGather latency hides under attention instead of stalling the
     in-order PE queue.
"""
import numpy as np
from contextlib import ExitStack

import concourse.bass as bass
import concourse.mybir as mybir
import concourse.tile as tile
import concourse.bacc as bacc
from concourse.bass_utils import run_bass_kernel_spmd

F32 = mybir.dt.float32
F32R = mybir.dt.float32r
BF16 = mybir.dt.bfloat16
AF = mybir.ActivationFunctionType
MUL = mybir.AluOpType.mult

B, S, D, H, DH = 2, 2048, 1024, 16, 64
EPS = 1e-5
N_CORES = 8
EL = 256            # e-dims per core
NHL = 4             # heads per core
TC = 512            # token chunk (matmul moving N)
NTC = S // TC       # 4
NDC = D // 128      # 8 contraction chunks
NJC = S // 128      # 16 key chunks
GROUPS = [[0, 1, 2, 3], [4, 5, 6, 7]]

TRACE = False       # test.py flips this for profiling


def _emit_rope(nc, tc, ab_pool, tmp_pool, psw, perm_sb, ra, rb, RSq, srct, rot, et, is_q):
    esl = slice(128 * et, 128 * et + 128)
    for tcix in range(NTC):
        tsl = slice(TC * tcix, TC * tcix + TC)
        at = ab_pool.tile([128, TC], F32, tag="ab", name="at")
        bt = ab_pool.tile([128, TC], F32, tag="ab", name="bt")
        nc.sync.dma_start(at[:], ra[esl, tsl])
        nc.sync.dma_start(bt[:], rb[esl, tsl])
        swp = psw.tile([128, TC], F32, tag="psw")
        nc.tensor.matmul(swp[:], perm_sb[:], srct[et][:, tsl], start=True, stop=True)
        t1 = tmp_pool.tile([128, TC], F32, tag="tmp")
        nc.vector.tensor_tensor(t1[:], at[:], srct[et][:, tsl], MUL)
        t2 = tmp_pool.tile([128, TC], F32, tag="tmp")
        nc.vector.tensor_tensor(t2[:], bt[:], swp[:], MUL)
        if is_q:
            t3 = tmp_pool.tile([128, TC], F32, tag="tmp")
            nc.vector.tensor_add(t3[:], t1[:], t2[:])
            nc.vector.tensor_tensor(rot[et][:, tsl], t3[:], RSq[:, tsl], MUL)
        else:
            nc.vector.tensor_add(rot[et][:, tsl], t1[:], t2[:])


def _emit(nc):
    xT = nc.declare_dram_parameter("xT", [D, S], BF16, isOutput=False)
    wq = nc.declare_dram_parameter("wq", [128, NDC * EL], BF16, isOutput=False)
    wk = nc.declare_dram_parameter("wk", [128, NDC * EL], BF16, isOutput=False)
    wv = nc.declare_dram_parameter("wv", [128, NDC * EL], BF16, isOutput=False)
    wo = nc.declare_dram_parameter("wo", [128, NDC * EL], BF16, isOutput=False)
    bq = nc.declare_dram_parameter("bq", [EL, 1], F32, isOutput=False)
    bk = nc.declare_dram_parameter("bk", [EL, 1], F32, isOutput=False)
    bv = nc.declare_dram_parameter("bv", [1, EL], BF16, isOutput=False)
    bo = nc.declare_dram_parameter("bo", [EL, 1], F32, isOutput=False)
    gq = nc.declare_dram_parameter("gq", [EL, 1], F32, isOutput=False)
    gk = nc.declare_dram_parameter("gk", [EL, 1], F32, isOutput=False)
    ra = nc.declare_dram_parameter("rope_a", [EL, S], F32, isOutput=False)
    rb = nc.declare_dram_parameter("rope_b", [EL, S], F32, isOutput=False)
    pm = nc.declare_dram_parameter("perm", [128, 128], F32, isOutput=False)
    yT = nc.declare_dram_parameter("yT", [EL, S], F32, isOutput=True)

    with tile.TileContext(nc) as tc, ExitStack() as ctx:
        # ---------------- persistent pools ----------------
        w_pool = ctx.enter_context(tc.tile_pool(name="w", bufs=1))
        x_pool = ctx.enter_context(tc.tile_pool(name="x", bufs=1))
        qk_pool = ctx.enter_context(tc.tile_pool(name="qk", bufs=4))
        v_pool = ctx.enter_context(tc.tile_pool(name="v", bufs=16))
        small = ctx.enter_context(tc.tile_pool(name="small", bufs=1))
        rsq_pool = ctx.enter_context(tc.tile_pool(name="rsq", bufs=1))
        dram = ctx.enter_context(tc.tile_pool(name="dram", bufs=16, space="DRAM"))

        # ---------------- CC warm-up: tiny AllReduce at t=0 ----------------
        warm_in = dram.tile([1, 8], F32, tag="warm")
        warm_out = dram.tile([1, 8], F32, tag="warm")
        wz = small.tile([1, 8], F32, tag="wz")
        nc.vector.memset(wz[:], 0.0)
        nc.sync.dma_start(warm_in[:], wz[:])
        nc.gpsimd.collective_compute(
            "AllReduce", mybir.AluOpType.add, replica_groups=GROUPS,
            ins=[warm_in[:].opt()], outs=[warm_out[:].opt()],
        )

        # ---------------- constants / small loads ----------------
        bq_sb, bk_sb, bo_sb, gq_sb, gk_sb = [], [], [], [], []
        for et in range(2):
            sl = slice(128 * et, 128 * et + 128)
            t = small.tile([128, 1], F32, tag=f"bq{et}", name=f"bq{et}")
            nc.sync.dma_start(t[:], bq[sl, :]); bq_sb.append(t)
            t = small.tile([128, 1], F32, tag=f"bk{et}", name=f"bk{et}")
            nc.sync.dma_start(t[:], bk[sl, :]); bk_sb.append(t)
            t = small.tile([128, 1], F32, tag=f"bo{et}", name=f"bo{et}")
            nc.sync.dma_start(t[:], bo[sl, :]); bo_sb.append(t)
            t = small.tile([128, 1], F32R, tag=f"gq{et}", name=f"gq{et}")
            nc.sync.dma_start(t[:], gq[sl, :].bitcast(F32R)); gq_sb.append(t)
            t = small.tile([128, 1], F32R, tag=f"gk{et}", name=f"gk{et}")
            nc.sync.dma_start(t[:], gk[sl, :].bitcast(F32R)); gk_sb.append(t)
        bv_sb = small.tile([1, EL], BF16, tag="bvrow")
        nc.sync.dma_start(bv_sb[:], bv[:])
        ones1 = small.tile([1, 128], F32, tag="ones1")
        nc.vector.memset(ones1[:], 1.0)
        ones1b = small.tile([1, 128], BF16, tag="ones1b")
        nc.vector.tensor_copy(ones1b[:], ones1[:])
        ones4 = small.tile([128, 4], F32, tag="ones4")
        nc.vector.memset(ones4[:], 1.0)
        ones41 = small.tile([4, 1], F32R, tag="ones41")
        nc.vector.tensor_copy(ones41[:], ones4[0:4, 0:1])
        eps_t = small.tile([2, 1], F32, tag="eps")
        nc.vector.memset(eps_t[:], EPS)
        perm_sb = small.tile([128, 128], F32R, tag="perm")
        nc.sync.dma_start(perm_sb[:], pm[:].bitcast(F32R))

        # ---------------- weights (bf16) + x (bf16, loaded ONCE) ----------
        wk_all = w_pool.tile([128, NDC * EL], BF16, tag="wk", name="wk_all", bufs=1)
        wq_all = w_pool.tile([128, NDC * EL], BF16, tag="wq", name="wq_all", bufs=1)
        wv_all = w_pool.tile([128, NDC * EL], BF16, tag="wv", name="wv_all", bufs=1)
        wo_all = w_pool.tile([128, NDC * EL], BF16, tag="wo", name="wo_all", bufs=1)
        wq_sb = [wq_all[:, EL * i:EL * i + EL] for i in range(NDC)]
        wk_sb = [wk_all[:, EL * i:EL * i + EL] for i in range(NDC)]
        wv_sb = [wv_all[:, EL * i:EL * i + EL] for i in range(NDC)]
        wo_sb = [wo_all[:, EL * i:EL * i + EL] for i in range(NDC)]
        nc.sync.dma_start(wk_all[:], wk[:])
        nc.sync.dma_start(wq_all[:], wq[:])
        nc.sync.dma_start(wv_all[:], wv[:])
        nc.sync.dma_start(wo_all[:], wo[:])

        xt = [x_pool.tile([128, S], BF16, tag=f"x{i}", name=f"xt{i}", bufs=1)
              for i in range(NDC)]
        for tcix in range(NTC):
            tsl = slice(TC * tcix, TC * tcix + TC)
            for dc in range(NDC):
                nc.sync.dma_start(xt[dc][:, tsl], xT[128 * dc:128 * dc + 128, tsl])

        # persistent activation tiles
        qT_sb = [qk_pool.tile([128, S], F32R, tag="qk", name=f"qT_sb{i}") for i in range(2)]
        kT_sb = [qk_pool.tile([128, S], F32R, tag="qk", name=f"kT_sb{i}") for i in range(2)]
        v_sb = [v_pool.tile([128, NHL * 65], BF16, tag="v", name=f"v_sb{i}") for i in range(NJC)]
        for jt in range(NJC):
            ones_cols = v_sb[jt][:].rearrange("p (h c) -> p h c", c=65)[:, :, 64:65]
            nc.vector.tensor_copy(
                ones_cols, ones4[:].rearrange("p (f o) -> p f o", o=1)
            )
        RSq = rsq_pool.tile([128, S], F32, tag="rsq")
        rskT = small.tile([128, NJC], F32, tag="rskT")

        # ---------------- phase A: projections + ssq + rope ----------------
        rot_q = [None, None]
        rot_k = [None, None]
        with ExitStack() as actx:
            sq_pool = actx.enter_context(tc.tile_pool(name="sq", bufs=2))
            ab_pool = actx.enter_context(tc.tile_pool(name="ab", bufs=4))
            tmp_pool = actx.enter_context(tc.tile_pool(name="tmp", bufs=6))
            rs_pool = actx.enter_context(tc.tile_pool(name="rs", bufs=4))
            pq = actx.enter_context(tc.tile_pool(name="pq", bufs=2, space="PSUM"))
            pv = actx.enter_context(tc.tile_pool(name="pv", bufs=2, space="PSUM"))
            pssq = actx.enter_context(tc.tile_pool(name="pssq", bufs=2, space="PSUM"))
            psw = actx.enter_context(tc.tile_pool(name="psw", bufs=2, space="PSUM"))

            ar_in = dram.tile([1, 2 * S], F32, tag="arin")
            ar_out = dram.tile([4, 2 * S], F32, tag="arout")

            # ---- K pass, then Q pass (each: e-major projection + ssq) ----
            # k ssq -> ar row 0, q ssq -> ar row 1; ONE combined AllReduce.
            for wsb, bsb, gsb, dst, arrow in (
                (wk_sb, bk_sb, gk_sb, kT_sb, 0),
                (wq_sb, bq_sb, gq_sb, qT_sb, 1),
            ):
                for tcix in range(NTC):
                    tsl = slice(TC * tcix, TC * tcix + TC)
                    ssq_ps = pssq.tile([1, TC], F32, tag="pssq")
                    for et in range(2):
                        esl = slice(128 * et, 128 * et + 128)
                        qp = pq.tile([128, TC], F32, tag="pq")
                        for dc in range(NDC):
                            nc.tensor.matmul(
                                qp[:], wsb[dc][:, esl], xt[dc][:, tsl],
                                start=(dc == 0), stop=(dc == NDC - 1),
                            )
                        nc.scalar.activation(
                            dst[et][:, tsl], qp[:], AF.Identity, bias=bsb[et][:]
                        )
                        sq = sq_pool.tile([128, TC], F32R, tag="sq")
                        nc.scalar.activation(sq[:], qp[:], AF.Square, bias=bsb[et][:])
                        nc.tensor.matmul(
                            ssq_ps[:], gsb[et][:], sq[:],
                            start=(et == 0), stop=(et == 1),
                        )
                    ssq_c = rs_pool.tile([1, TC], F32, tag="rs", name="ssq_c")
                    nc.vector.tensor_copy(ssq_c[:], ssq_ps[:])
                    nc.sync.dma_start(
                        ar_in[0:1, arrow * S + TC * tcix:arrow * S + TC * tcix + TC],
                        ssq_c[:],
                    )
            nc.gpsimd.collective_compute(
                "AllGather", mybir.AluOpType.bypass, replica_groups=GROUPS,
                ins=[ar_in[:].opt()], outs=[ar_out[:].opt()],
            )

            # ---- k RoPE (no AR dependency) emitted before V so DVE overlaps it ----
            for et in range(2):
                rot_q[et] = qk_pool.tile([128, S], BF16, tag="rot", name=f"rot_q{et}")
                rot_k[et] = qk_pool.tile([128, S], BF16, tag="rot", name=f"rot_k{et}")
            for et in range(2):
                _emit_rope(nc, tc, ab_pool, tmp_pool, psw, perm_sb, ra, rb, RSq,
                           kT_sb, rot_k, et, False)

            # ---- V pass (t-major) ----
            for tcix in range(NTC):
                for ts in range(4):
                    jt = 4 * tcix + ts
                    vsl = slice(TC * tcix + 128 * ts, TC * tcix + 128 * ts + 128)
                    vp = pv.tile([128, EL], F32, tag="pv")
                    for dc in range(NDC):
                        nc.tensor.matmul(
                            vp[:], xt[dc][:, vsl], wv_sb[dc][:],
                            start=(dc == 0), stop=False,
                        )
                    nc.tensor.matmul(vp[:], ones1b[:], bv_sb[:], start=False, stop=True)
                    nc.scalar.activation(
                        v_sb[jt][:].rearrange("p (h c) -> p h c", c=65)[:, :, 0:64],
                        vp[:].rearrange("p (h c) -> p h c", c=64),
                        AF.Identity,
                    )

            # ---- local reduction of the gathered ssq partials ----
            # ar_out: [4 cores, 2S] with k in cols 0:S, q in cols S:2S.
            # Sum over the 4 rows via a K=4 ones-stationary matmul per chunk.
            g_sb = small.tile([4, 2 * S], F32R, tag="gsb")
            nc.sync.dma_start(g_sb[:], ar_out[:].bitcast(F32R))
            # q chunks first: rs_q feeds rope_q (critical path)
            qstd = rs_pool.tile([1, S], F32, tag="qstd", bufs=1)
            for tcix in range(NTC):
                tsl = slice(TC * tcix, TC * tcix + TC)
                sp = pssq.tile([1, TC], F32, tag="pssq", name="spq")
                nc.tensor.matmul(sp[:], ones41[:],
                                 g_sb[:, S + TC * tcix:S + TC * tcix + TC],
                                 start=True, stop=True)
                nc.scalar.activation(qstd[0:1, tsl], sp[:], AF.Sqrt,
                                     scale=1.0 / D, bias=eps_t[0:1, :])
                rsq = rs_pool.tile([1, TC], F32, tag="rs", name="rsq")
                nc.vector.reciprocal(rsq[:], qstd[0:1, tsl])
                nc.gpsimd.partition_broadcast(RSq[:, tsl], rsq[0:1, :],
                                              channels=128)
            for et in range(2):
                _emit_rope(nc, tc, ab_pool, tmp_pool, psw, perm_sb, ra, rb, RSq,
                           qT_sb, rot_q, et, True)
            # k chunks: std_k row, transposed to columns via K=1 matmuls, then
            # one lane-parallel reciprocal over [128, NJC]
            kstd = rs_pool.tile([1, S], F32, tag="kstd", bufs=1)
            for tcix in range(NTC):
                tsl = slice(TC * tcix, TC * tcix + TC)
                sp = pssq.tile([1, TC], F32, tag="pssq", name="spk")
                nc.tensor.matmul(sp[:], ones41[:],
                                 g_sb[:, TC * tcix:TC * tcix + TC],
                                 start=True, stop=True)
                nc.scalar.activation(kstd[0:1, tsl], sp[:], AF.Sqrt,
                                     scale=1.0 / D, bias=eps_t[0:1, :])
            pT = psw.tile([128, NJC], F32, tag="psw", name="pT")
            for jc in range(NJC):
                nc.tensor.matmul(
                    pT[:, jc:jc + 1], kstd[0:1, 128 * jc:128 * jc + 128],
                    ones1[0:1, 0:1], start=True, stop=True,
                )
            recT = small.tile([128, NJC], F32, tag="recT")
            nc.vector.reciprocal(recT[:], pT[:])
            nc.scalar.mul(rskT[:], recT[:], 1.0 / float(np.sqrt(DH)))

        # ---------------- phase B: attention + AG + out-proj ----------------
        with ExitStack() as bctx:
            exp_pool = bctx.enter_context(tc.tile_pool(name="exp", bufs=3))
            og_pool = bctx.enter_context(tc.tile_pool(name="og", bufs=8))
            dv_pool = bctx.enter_context(tc.tile_pool(name="dv", bufs=4))
            y_pool = bctx.enter_context(tc.tile_pool(name="y", bufs=2))
            rec_pool = bctx.enter_context(tc.tile_pool(name="rec", bufs=2))
            psc = bctx.enter_context(tc.tile_pool(name="psc", bufs=2, space="PSUM"))
            pav = bctx.enter_context(tc.tile_pool(name="pav", bufs=3, space="PSUM"))
            py = bctx.enter_context(tc.tile_pool(name="py", bufs=1, space="PSUM"))

            def emit_outproj(ag_out, isl):
                og = [og_pool.tile([128, TC], BF16, tag="og", name=f"og{i}")
                      for i in range(NDC)]
                for ec in range(NDC):
                    nc.sync.dma_start(og[ec][:], ag_out[128 * ec:128 * ec + 128, :])
                for dt2 in range(2):
                    dsl = slice(128 * dt2, 128 * dt2 + 128)
                    yp = py.tile([128, TC], F32, tag="py")
                    for ec in range(NDC):
                        nc.tensor.matmul(
                            yp[:], wo_sb[ec][:, dsl], og[ec][:],
                            start=(ec == 0), stop=(ec == NDC - 1),
                        )
                    ys = y_pool.tile([128, TC], F32, tag="y")
                    nc.scalar.activation(ys[:], yp[:], AF.Identity, bias=bo_sb[dt2][:])
                    nc.sync.dma_start(yT[dsl, isl], ys[:])

            prev = None                       # (ag_out, isl) of the previous i-tile
            for itix in range(NTC):           # i-tile loop (queries)
                isl = slice(TC * itix, TC * itix + TC)
                ag_in = dram.tile([EL, TC], BF16, tag="agin")
                avs_all = []
                for hp in range(2):           # head pair
                    q_t, k_t = rot_q[hp], rot_k[hp]
                    av = [pav.tile([65, TC], F32, tag="pav", name=f"av{i}") for i in range(2)]
                    for jc in range(NJC):
                        jsl = slice(128 * jc, 128 * jc + 128)
                        sc = psc.tile([128, 2 * TC], F32, tag="psc")
                        for hh in range(2):
                            psl = slice(64 * hh, 64 * hh + 64)
                            nc.tensor.matmul(
                                sc[:, TC * hh:TC * hh + TC],
                                k_t[psl, jsl], q_t[psl, isl],
                                start=True, stop=True,
                            )
                        ex = exp_pool.tile([128, 2 * TC], BF16, tag="exp")
                        nc.scalar.activation(
                            ex[:], sc[:], AF.Exp, scale=rskT[:, jc:jc + 1]
                        )
                        for hh in range(2):
                            h4 = 2 * hp + hh
                            nc.tensor.matmul(
                                av[hh][:],
                                v_sb[jc][:, 65 * h4:65 * h4 + 65],
                                ex[:, TC * hh:TC * hh + TC],
                                start=(jc == 0), stop=(jc == NJC - 1),
                            )
                    for hh in range(2):
                        avs = dv_pool.tile([65, TC], F32, tag="avs")
                        nc.vector.tensor_copy(avs[:], av[hh][:])
                        avs_all.append(avs)
                # reciprocals AFTER all four PSUM-freeing copies, so the
                # next i-tile's AV accumulators never wait on the DVE
                recs = []
                for i in range(4):
                    rec = rec_pool.tile([1, TC], F32, tag="rec", bufs=4)
                    nc.vector.reciprocal(rec[:], avs_all[i][64:65, :])
                    recs.append(rec)
                for i in range(4):
                    rcb = rec_pool.tile([64, TC], F32, tag="recb")
                    nc.gpsimd.partition_broadcast(rcb[:], recs[i][0:1, :], channels=64)
                    dv = dv_pool.tile([64, TC], BF16, tag="dv")
                    nc.vector.tensor_tensor(dv[:], avs_all[i][0:64, :], rcb[:], MUL)
                    nc.sync.dma_start(ag_in[64 * i:64 * i + 64, :], dv[:])
                # AllGather this i-tile's outputs across the batch group
                ag_out = dram.tile([4 * EL, TC], BF16, tag="agout")
                nc.gpsimd.collective_compute(
                    "AllGather", mybir.AluOpType.bypass, replica_groups=GROUPS,
                    ins=[ag_in[:].opt()], outs=[ag_out[:].opt()],
                )
                # out-proj for the PREVIOUS i-tile: its AllGather completed
                # while this i-tile's attention was running, so the PE never
                # waits on a collective.
                if prev is not None:
                    emit_outproj(*prev)
                prev = (ag_out, isl)
            emit_outproj(*prev)


def build_nc():
    nc = bacc.Bacc("TRN2", target_bir_lowering=False, debug=False, num_devices=N_CORES)
    _emit(nc)
    nc.compile()
    return nc


_NC_CACHE = None


def _get_nc():
    global _NC_CACHE
    if _NC_CACHE is None:
        _NC_CACHE = build_nc()
    return _NC_CACHE


def _host_prep(inputs):
    import ml_dtypes
    bf16 = ml_dtypes.bfloat16

    x = np.ascontiguousarray(np.asarray(inputs["x"], dtype=np.float32))
    pe = np.asarray(inputs["pe"], dtype=np.float32)[0, 0]      # [S, D//2, 2, 2]
    Wq = np.asarray(inputs["Wq"], dtype=np.float32)
    bq = np.asarray(inputs["bq"], dtype=np.float32)
    Wk = np.asarray(inputs["Wk"], dtype=np.float32)
    bk = np.asarray(inputs["bk"], dtype=np.float32)
    Wv = np.asarray(inputs["Wv"], dtype=np.float32)
    bv = np.asarray(inputs["bv"], dtype=np.float32)
    qn = np.asarray(inputs["qn_w"], dtype=np.float32)
    kn = np.asarray(inputs["kn_w"], dtype=np.float32)
    Wo = np.asarray(inputs["Wo"], dtype=np.float32)
    bo = np.asarray(inputs["bo"], dtype=np.float32)

    Wq_f = Wq * qn[:, None]
    bq_f = qn * bq
    Wk_f = Wk * kn[:, None]
    bk_f = kn * bk

    A = np.empty((D, S), np.float32)
    Bm = np.empty((D, S), np.float32)
    A[0::2, :] = pe[:, :, 0, 0].T
    A[1::2, :] = pe[:, :, 1, 1].T
    Bm[0::2, :] = pe[:, :, 0, 1].T
    Bm[1::2, :] = pe[:, :, 1, 0].T

    perm = np.zeros((128, 128), np.float32)
    idx = np.arange(64)
    perm[2 * idx, 2 * idx + 1] = 1.0
    perm[2 * idx + 1, 2 * idx] = 1.0

    def pack_dT(m):          # m: [D, F] -> [128, NDC*F] (dc-major wide rows)
        F = m.shape[1]
        return np.ascontiguousarray(
            m.reshape(NDC, 128, F).transpose(1, 0, 2).reshape(128, NDC * F)
        ).astype(bf16)
    xTs = [np.ascontiguousarray(x[b].T).astype(bf16) for b in range(B)]
    in_maps = []
    for c in range(N_CORES):
        b, g = c // 4, c % 4
        E = slice(256 * g, 256 * g + 256)
        in_maps.append({
            "xT": xTs[b],
            "wq": pack_dT(Wq_f[E, :].T.copy()),
            "wk": pack_dT(Wk_f[E, :].T.copy()),
            "wv": pack_dT(Wv[E, :].T.copy()),
            "wo": pack_dT(Wo[E, :].T.copy()),
            "bq": np.ascontiguousarray(bq_f[E])[:, None],
            "bk": np.ascontiguousarray(bk_f[E])[:, None],
            "bv": np.ascontiguousarray(bv[E])[None, :].astype(bf16),
            "bo": np.ascontiguousarray(bo[E])[:, None],
            "gq": np.ascontiguousarray(1.0 / qn[E] ** 2)[:, None],
            "gk": np.ascontiguousarray(1.0 / kn[E] ** 2)[:, None],
            "rope_a": np.ascontiguousarray(A[E, :]),
            "rope_b": np.ascontiguousarray(Bm[E, :]),
            "perm": perm,
        })
    return in_maps


def kernel(**inputs):
    nc = _get_nc()
    in_maps = _host_prep(inputs)
    res = run_bass_kernel_spmd(
        nc, in_maps, core_ids=list(range(N_CORES)), trace=TRACE
    )
    if TRACE and res.exec_time_ns is not None:
        print(f"HW exec time: {res.exec_time_ns} ns")
    y = np.empty((B, S, D), np.float32)
    for c in range(N_CORES):
        b, g = c // 4, c % 4
        y[b][:, 256 * g:256 * g + 256] = res.results[c]["yT"].T
    return y
